# revision 4
# baseline (speedup 1.0000x reference)
"""Causal self-attention kernel for 8 trn2 NeuronCores.

Sharding: core c handles batch b = c // 4 and local head group hg = c % 4
(4 of the 16 heads). Tensor-parallel over heads for kqv / attention and
row-parallel for the output projection; the 4 per-batch partial projections
are summed on the host (the "all-reduce" of classic TP), where the bias is
also added.

Device kernel (per core, bf16 matmuls, fp32 accumulation), interleaved over
tq windows g of 512 so PE / ScalarE / VectorE / DMA overlap end to end:
  kq(g):   kqT window = (Wqk x^T)[:, g]      [512 feat, 512 t]  k,q head-major
  v(g):    v chunks 4g..4g+3 = (x Wv^T)      per-head [128 t, 64] + 64-wide
           ones block (used to compute softmax denominators on the PE)
  attn(g): per head (paired for PE row-group overlap), tk chunks j <= 4g+3:
             S^T = k^T.T q^T   (K=64, heads at partition 0/64 run concurrently)
             P = exp(S^T/8) on ScalarE (no max subtraction; scores are O(1))
             diagonal causal mask = multiply by static triangle on VectorE
             O^T psum[0:64] += v_j.T @ P ; psum[64:128] += ones.T @ P (=denom)
           normalize: reciprocal_approx_fast + multiply on VectorE
  proj(g-1): y[:, window] = O_cat^T.T @ Wp^T -> fp32, DMA out (one window
           deferred so the PE never waits on the normalize chain)
"""

import numpy as np
import ml_dtypes

T = 2048
C = 1024
NH_LOCAL = 4
D = 64
TQW = 512  # tq window width
NCHUNK = T // 128  # 16 tk chunks
NGRP = T // TQW  # 4 tq windows

_nc_cache = {}


def _build_bass():
    import concourse.mybir as mybir
    import concourse.tile as tile
    from concourse import bacc

    f32 = mybir.dt.float32
    bf16 = mybir.dt.bfloat16

    nc = bacc.Bacc(None, target_bir_lowering=False)
    xt_d = nc.dram_tensor("xt", [C, T], bf16, kind="ExternalInput")
    wqk_d = nc.dram_tensor("wqk", [C, 512], bf16, kind="ExternalInput")
    wv_d = nc.dram_tensor("wv", [C, 256], bf16, kind="ExternalInput")
    wp_d = nc.dram_tensor("wp", [256, C], bf16, kind="ExternalInput")
    y_d = nc.dram_tensor("y", [T, C], f32, kind="ExternalOutput")

    with tile.TileContext(nc) as tc:
        with (
            tc.tile_pool(name="persist", bufs=1) as pp,
            tc.tile_pool(name="mmp", bufs=2, space="PSUM") as mp,
            tc.tile_pool(name="spsum", bufs=4, space="PSUM") as sp,
            tc.tile_pool(name="opsum", bufs=1, space="PSUM") as op,
            tc.tile_pool(name="ptp", bufs=6) as ptp,
            tc.tile_pool(name="rp", bufs=2) as rp,
            tc.tile_pool(name="ysb", bufs=2) as ysb,
        ):
            xt_s = [pp.tile([128, T], bf16, tag=f"xt{c}", name=f"xt{c}") for c in range(8)]
            wqk_s = [pp.tile([128, 512], bf16, tag=f"wqk{c}", name=f"wqk{c}") for c in range(8)]
            wv_s = [pp.tile([128, 256], bf16, tag=f"wv{c}", name=f"wv{c}") for c in range(8)]
            wp_s = [pp.tile([128, C], bf16, tag=f"wp{c}", name=f"wp{c}") for c in range(2)]
            kq_s = [pp.tile([128, T], bf16, tag=f"kq{f}", name=f"kq{f}") for f in range(4)]
            v_s = [pp.tile([128, T], bf16, tag=f"v{h}", name=f"v{h}") for h in range(NH_LOCAL)]
            oc_s = [pp.tile([128, T], bf16, tag=f"oc{p}", name=f"oc{p}") for p in range(2)]
            m_s = pp.tile([128, 128], bf16, tag="mask", name="mask")

            # weights first (small), then x windows in g-order so kq(g=0) can
            # start while later x chunks stream in
            for c in range(8):
                nc.sync.dma_start(wqk_s[c][:], wqk_d[128 * c : 128 * (c + 1), :])
                nc.sync.dma_start(wv_s[c][:], wv_d[128 * c : 128 * (c + 1), :])
            for p in range(2):
                nc.sync.dma_start(wp_s[p][:], wp_d[128 * p : 128 * (p + 1), :])
            for g in range(NGRP):
                for c in range(8):
                    nc.sync.dma_start(
                        xt_s[c][:, TQW * g : TQW * (g + 1)],
                        xt_d[128 * c : 128 * (c + 1), TQW * g : TQW * (g + 1)],
                    )

            # causal triangle (1 where col >= partition else 0), built once
            nc.vector.memset(m_s[:], 1.0)
            nc.gpsimd.affine_select(
                out=m_s[:],
                in_=m_s[:],
                compare_op=mybir.AluOpType.is_ge,
                fill=0.0,
                base=0,
                pattern=[[1, 128]],
                channel_multiplier=-1,
            )
            # ones blocks for the denominator trick (v regions overwritten later)
            for h in range(NH_LOCAL):
                nc.vector.memset(v_s[h][:], 1.0)

            def proj(g):
                for i in range(4 * g, 4 * g + 4):
                    ys = ysb.tile([128, C], f32, tag="ys", name="ys")
                    for u in range(2):
                        acc = mp.tile([128, 512], f32, tag="mm", name="mmy")
                        for ci in range(2):
                            nc.tensor.matmul(
                                acc[:],
                                oc_s[ci][:, 128 * i : 128 * (i + 1)],
                                wp_s[ci][:, 512 * u : 512 * (u + 1)],
                                start=(ci == 0),
                                stop=(ci == 1),
                            )
                        nc.vector.tensor_copy(ys[:, 512 * u : 512 * (u + 1)], acc[:])
                    nc.sync.dma_start(y_d[128 * i : 128 * (i + 1), :], ys[:])

            for g in range(NGRP):
                # ---- kq for window g ----
                for f in range(4):
                    acc = mp.tile([128, TQW], f32, tag="mm", name="mmkq")
                    for c in range(8):
                        nc.tensor.matmul(
                            acc[:],
                            wqk_s[c][:, 128 * f : 128 * (f + 1)],
                            xt_s[c][:, TQW * g : TQW * (g + 1)],
                            start=(c == 0),
                            stop=(c == 7),
                        )
                    nc.vector.tensor_copy(kq_s[f][:, TQW * g : TQW * (g + 1)], acc[:])

                # ---- v chunks 4g..4g+3 ----
                for j in range(4 * g, 4 * g + 4):
                    acc = mp.tile([128, TQW], f32, tag="mm", name="mmv")
                    for c in range(8):
                        nc.tensor.matmul(
                            acc[:, :256],
                            xt_s[c][:, 128 * j : 128 * (j + 1)],
                            wv_s[c][:],
                            start=(c == 0),
                            stop=(c == 7),
                        )
                    for h in range(NH_LOCAL):
                        nc.vector.tensor_copy(
                            v_s[h][:, 128 * j : 128 * j + 64],
                            acc[:, 64 * h : 64 * (h + 1)],
                        )

                # ---- attention for window g, heads in pairs ----
                jmax = 4 * g + 3
                for hp in range(2):
                    pair = (2 * hp, 2 * hp + 1)
                    o_t = {}
                    for h in pair:
                        o_t[h] = op.tile([128, TQW], f32, tag=f"o{h % 2}", name=f"o{h % 2}")
                    for j in range(jmax + 1):
                        cs = max(0, 128 * j - TQW * g)
                        n = TQW - cs
                        s_t, pt = {}, {}
                        for h in pair:
                            kT = kq_s[h // 2][64 * (h % 2) : 64 * (h % 2) + 64, :]
                            qT = kq_s[2 + h // 2][64 * (h % 2) : 64 * (h % 2) + 64, :]
                            s_t[h] = sp.tile([128, TQW], f32, tag="s", name="s")
                            nc.tensor.matmul(
                                s_t[h][:, :n],
                                kT[:, 128 * j : 128 * (j + 1)],
                                qT[:, TQW * g + cs : TQW * (g + 1)],
                                start=True,
                                stop=True,
                            )
                        for h in pair:
                            pt[h] = ptp.tile([128, TQW], bf16, tag="pt", name="pt")
                            nc.scalar.activation(
                                pt[h][:, :n],
                                s_t[h][:, :n],
                                mybir.ActivationFunctionType.Exp,
                                scale=float(D) ** -0.5,
                            )
                            if j >= 4 * g:
                                nc.vector.tensor_tensor(
                                    pt[h][:, 0:128],
                                    pt[h][:, 0:128],
                                    m_s[:],
                                    mybir.AluOpType.mult,
                                )
                        for h in pair:
                            nc.tensor.matmul(
                                o_t[h][:, cs:TQW],
                                v_s[h][:, 128 * j : 128 * (j + 1)],
                                pt[h][:, :n],
                                start=(j == 0),
                                stop=(j == jmax),
                            )
                    for h in pair:
                        lsb = rp.tile([64, TQW], f32, tag="lsb", name="lsb")
                        rinv = rp.tile([64, TQW], f32, tag="rinv", name="rinv")
                        nc.vector.tensor_copy(lsb[:], o_t[h][64:128, :])
                        nc.vector.reciprocal_approx_fast(rinv[:], lsb[:])
                        nc.vector.tensor_tensor(
                            oc_s[h // 2][
                                64 * (h % 2) : 64 * (h % 2) + 64,
                                TQW * g : TQW * (g + 1),
                            ],
                            o_t[h][0:64, :],
                            rinv[:],
                            mybir.AluOpType.mult,
                        )

                # ---- projection for the previous window ----
                if g > 0:
                    proj(g - 1)
            proj(NGRP - 1)

    nc.compile()
    return nc


def get_nc():
    if "nc" not in _nc_cache:
        _nc_cache["nc"] = _build_bass()
    return _nc_cache["nc"]


def _shard_inputs(x, W_kqv, W_proj):
    """Build the 8 per-core input maps (host-side shard + transpose + cast)."""
    bf16 = ml_dtypes.bfloat16
    in_maps = []
    for core in range(8):
        b, hg = core // 4, core % 4
        heads = range(4 * hg, 4 * hg + 4)
        xt = np.ascontiguousarray(x[b].T).astype(bf16)  # [C, T]
        k_rows = [W_kqv[64 * h : 64 * (h + 1)] for h in heads]
        q_rows = [W_kqv[C + 64 * h : C + 64 * (h + 1)] for h in heads]
        v_rows = [W_kqv[2 * C + 64 * h : 2 * C + 64 * (h + 1)] for h in heads]
        wqk = np.ascontiguousarray(np.concatenate(k_rows + q_rows, 0).T).astype(bf16)
        wv = np.ascontiguousarray(np.concatenate(v_rows, 0).T).astype(bf16)
        wp = np.ascontiguousarray(W_proj[:, 256 * hg : 256 * (hg + 1)].T).astype(bf16)
        in_maps.append({"xt": xt, "wqk": wqk, "wv": wv, "wp": wp})
    return in_maps


def kernel(x, W_kqv, W_proj, b_proj):
    from concourse.bass_utils import run_bass_kernel_spmd

    nc = get_nc()
    in_maps = _shard_inputs(x, W_kqv, W_proj)
    res = run_bass_kernel_spmd(nc, in_maps, core_ids=list(range(8)))
    B = x.shape[0]
    out = np.empty((B, T, C), np.float32)
    for b in range(B):
        acc = res.results[4 * b]["y"].astype(np.float32).copy()
        for hg in range(1, 4):
            acc += res.results[4 * b + hg]["y"]
        out[b] = acc + b_proj[None, :]
    return out


# revision 5
# speedup vs baseline: 1.0100x; 1.0100x over previous
"""Causal self-attention kernel for 8 trn2 NeuronCores.

Sharding: core c handles batch b = c // 4 and local head group hg = c % 4
(4 of the 16 heads). Tensor-parallel over heads for kqv / attention and
row-parallel for the output projection; the 4 per-batch partial projections
are summed on the host (the "all-reduce" of classic TP), where the bias is
also added.

Device kernel (per core, bf16 matmuls, fp32 accumulation), interleaved over
tq super-windows G of 1024 so PE / ScalarE / VectorE / DMA overlap end to end:
  kq(G):   kqT windows = (Wqk x^T)[:, G]     [512 feat, 1024 t]  k,q head-major
  v(G):    v chunks 8G..8G+7 = (x Wv^T)      per-head [128 t, 64] + 64-wide
           ones block (used to compute softmax denominators on the PE)
  attn(G): per head, tk chunks j <= 8G+7 (causally trimmed at col cs):
             S^T = k^T.T q^T into a 2-bank psum strip   (K=64)
             P = exp(S^T/8): ONE ScalarE activation over up to 1024 cols
             (no max subtraction -- scores are O(1) by construction)
             diagonal causal mask: affine_select on GpSimd (128x128 square)
             O^T psum[0:64] += v_j.T @ P ; psum[64:128] += ones.T @ P (=denom)
             per 512-wide sub-window accumulators
           normalize: l -> SBUF, reciprocal_approx_fast, multiply (VectorE)
  proj(G-1): y[:, windows] = O_cat^T.T @ Wp^T -> fp32, DMA out (deferred one
           super-window so the PE never waits on the normalize chain)
"""

import numpy as np
import ml_dtypes

T = 2048
C = 1024
NH_LOCAL = 4
D = 64
GW = 1024  # tq super-window width
NG = T // GW  # 2 super-windows
NCHUNK = T // 128  # 16 tk chunks

_nc_cache = {}


def _build_bass():
    import concourse.mybir as mybir
    import concourse.tile as tile
    from concourse import bacc

    f32 = mybir.dt.float32
    bf16 = mybir.dt.bfloat16

    nc = bacc.Bacc(None, target_bir_lowering=False)
    xt_d = nc.dram_tensor("xt", [C, T], bf16, kind="ExternalInput")
    wqk_d = nc.dram_tensor("wqk", [C, 512], bf16, kind="ExternalInput")
    wv_d = nc.dram_tensor("wv", [C, 256], bf16, kind="ExternalInput")
    wp_d = nc.dram_tensor("wp", [256, C], bf16, kind="ExternalInput")
    y_d = nc.dram_tensor("y", [T, C], f32, kind="ExternalOutput")

    with tile.TileContext(nc) as tc:
        with (
            tc.tile_pool(name="persist", bufs=1) as pp,
            tc.tile_pool(name="mmp", bufs=2, space="PSUM") as mp,
            tc.tile_pool(name="spsum", bufs=2, space="PSUM") as sp,
            tc.tile_pool(name="opsum", bufs=1, space="PSUM") as op,
            tc.tile_pool(name="ptp", bufs=3) as ptp,
            tc.tile_pool(name="rp", bufs=4) as rp,
            tc.tile_pool(name="ysb", bufs=2) as ysb,
        ):
            xt_s = [pp.tile([128, T], bf16, tag=f"xt{c}", name=f"xt{c}") for c in range(8)]
            wqk_s = [pp.tile([128, 512], bf16, tag=f"wqk{c}", name=f"wqk{c}") for c in range(8)]
            wv_s = [pp.tile([128, 256], bf16, tag=f"wv{c}", name=f"wv{c}") for c in range(8)]
            wp_s = [pp.tile([128, C], bf16, tag=f"wp{c}", name=f"wp{c}") for c in range(2)]
            kq_s = [pp.tile([128, T], bf16, tag=f"kq{f}", name=f"kq{f}") for f in range(4)]
            v_s = [pp.tile([128, T], bf16, tag=f"v{h}", name=f"v{h}") for h in range(NH_LOCAL)]
            oc_s = [pp.tile([128, T], bf16, tag=f"oc{p}", name=f"oc{p}") for p in range(2)]

            # x(G=0) + qk weights first so kq(0) starts ASAP; wp last (needed
            # only at proj(0), deep into the kernel)
            for c in range(8):
                nc.sync.dma_start(wqk_s[c][:], wqk_d[128 * c : 128 * (c + 1), :])
            for c in range(8):
                nc.sync.dma_start(
                    xt_s[c][:, 0:GW], xt_d[128 * c : 128 * (c + 1), 0:GW]
                )
            for c in range(8):
                nc.sync.dma_start(wv_s[c][:], wv_d[128 * c : 128 * (c + 1), :])
            for c in range(8):
                nc.sync.dma_start(
                    xt_s[c][:, GW:T], xt_d[128 * c : 128 * (c + 1), GW:T]
                )
            for p in range(2):
                nc.sync.dma_start(wp_s[p][:], wp_d[128 * p : 128 * (p + 1), :])

            # ones blocks for the denominator trick (v regions overwritten later)
            for h in range(NH_LOCAL):
                nc.vector.memset(v_s[h][:], 1.0)

            def proj(gsup):
                for i in range(8 * gsup, 8 * gsup + 8):
                    ys = ysb.tile([128, C], f32, tag="ys", name="ys")
                    for u in range(2):
                        acc = mp.tile([128, 512], f32, tag="mm", name="mmy")
                        for ci in range(2):
                            nc.tensor.matmul(
                                acc[:],
                                oc_s[ci][:, 128 * i : 128 * (i + 1)],
                                wp_s[ci][:, 512 * u : 512 * (u + 1)],
                                start=(ci == 0),
                                stop=(ci == 1),
                            )
                        nc.vector.tensor_copy(ys[:, 512 * u : 512 * (u + 1)], acc[:])
                    nc.sync.dma_start(y_d[128 * i : 128 * (i + 1), :], ys[:])

            for G in range(NG):
                # ---- kq for super-window G (two 512 windows) ----
                for f in range(4):
                    for gg in range(2):
                        w0 = GW * G + 512 * gg
                        acc = mp.tile([128, 512], f32, tag="mm", name="mmkq")
                        for c in range(8):
                            nc.tensor.matmul(
                                acc[:],
                                wqk_s[c][:, 128 * f : 128 * (f + 1)],
                                xt_s[c][:, w0 : w0 + 512],
                                start=(c == 0),
                                stop=(c == 7),
                            )
                        nc.vector.tensor_copy(kq_s[f][:, w0 : w0 + 512], acc[:])

                # ---- v chunks 8G..8G+7 ----
                for j in range(8 * G, 8 * G + 8):
                    acc = mp.tile([128, 512], f32, tag="mm", name="mmv")
                    for c in range(8):
                        nc.tensor.matmul(
                            acc[:, :256],
                            xt_s[c][:, 128 * j : 128 * (j + 1)],
                            wv_s[c][:],
                            start=(c == 0),
                            stop=(c == 7),
                        )
                    for h in range(NH_LOCAL):
                        nc.vector.tensor_copy(
                            v_s[h][:, 128 * j : 128 * j + 64],
                            acc[:, 64 * h : 64 * (h + 1)],
                        )

                # ---- attention for super-window G ----
                jmax = 8 * G + 7  # last tk chunk (sub-window 1)
                jmax0 = 8 * G + 3  # last tk chunk reaching sub-window 0
                for h in range(NH_LOCAL):
                    kT = kq_s[h // 2][64 * (h % 2) : 64 * (h % 2) + 64, :]
                    qT = kq_s[2 + h // 2][64 * (h % 2) : 64 * (h % 2) + 64, :]
                    o0 = op.tile([128, 512], f32, tag="og0", name="og0")
                    o1 = op.tile([128, 512], f32, tag="og1", name="og1")
                    for j in range(jmax + 1):
                        cs = max(0, 128 * j - GW * G)
                        s_t = sp.tile([128, GW], f32, tag="s", name="s")
                        # S^T for both 512 sub-windows (same stationary kT_j)
                        if cs < 512:
                            nc.tensor.matmul(
                                s_t[:, cs:512],
                                kT[:, 128 * j : 128 * (j + 1)],
                                qT[:, GW * G + cs : GW * G + 512],
                                start=True,
                                stop=True,
                            )
                        cs1 = max(cs, 512)
                        nc.tensor.matmul(
                            s_t[:, cs1:GW],
                            kT[:, 128 * j : 128 * (j + 1)],
                            qT[:, GW * G + cs1 : GW * (G + 1)],
                            start=True,
                            stop=True,
                        )
                        pt = ptp.tile([128, GW], bf16, tag="pt", name="pt")
                        nc.scalar.activation(
                            pt[:, cs:GW],
                            s_t[:, cs:GW],
                            mybir.ActivationFunctionType.Exp,
                            scale=float(D) ** -0.5,
                        )
                        if 128 * j >= GW * G:
                            # diagonal square: keep col >= partition
                            nc.gpsimd.affine_select(
                                out=pt[:, cs : cs + 128],
                                in_=pt[:, cs : cs + 128],
                                compare_op=mybir.AluOpType.is_ge,
                                fill=0.0,
                                base=0,
                                pattern=[[1, 128]],
                                channel_multiplier=-1,
                            )
                        if cs < 512:
                            nc.tensor.matmul(
                                o0[:, cs:512],
                                v_s[h][:, 128 * j : 128 * (j + 1)],
                                pt[:, cs:512],
                                start=(j == 0),
                                stop=(j == jmax0),
                            )
                        nc.tensor.matmul(
                            o1[:, cs1 - 512 : 512],
                            v_s[h][:, 128 * j : 128 * (j + 1)],
                            pt[:, cs1:GW],
                            start=(j == 0),
                            stop=(j == jmax),
                        )
                    for sub, o_t in ((0, o0), (1, o1)):
                        w0 = GW * G + 512 * sub
                        lsb = rp.tile([64, 512], f32, tag="lsb", name="lsb")
                        rinv = rp.tile([64, 512], f32, tag="rinv", name="rinv")
                        nc.vector.tensor_copy(lsb[:], o_t[64:128, :])
                        nc.vector.reciprocal_approx_fast(rinv[:], lsb[:])
                        nc.vector.tensor_tensor(
                            oc_s[h // 2][
                                64 * (h % 2) : 64 * (h % 2) + 64, w0 : w0 + 512
                            ],
                            o_t[0:64, :],
                            rinv[:],
                            mybir.AluOpType.mult,
                        )

                # ---- projection for the previous super-window ----
                if G > 0:
                    proj(G - 1)
            proj(NG - 1)

    nc.compile()
    return nc


def get_nc():
    if "nc" not in _nc_cache:
        _nc_cache["nc"] = _build_bass()
    return _nc_cache["nc"]


def _shard_inputs(x, W_kqv, W_proj):
    """Build the 8 per-core input maps (host-side shard + transpose + cast)."""
    bf16 = ml_dtypes.bfloat16
    in_maps = []
    for core in range(8):
        b, hg = core // 4, core % 4
        heads = range(4 * hg, 4 * hg + 4)
        xt = np.ascontiguousarray(x[b].T).astype(bf16)  # [C, T]
        k_rows = [W_kqv[64 * h : 64 * (h + 1)] for h in heads]
        q_rows = [W_kqv[C + 64 * h : C + 64 * (h + 1)] for h in heads]
        v_rows = [W_kqv[2 * C + 64 * h : 2 * C + 64 * (h + 1)] for h in heads]
        wqk = np.ascontiguousarray(np.concatenate(k_rows + q_rows, 0).T).astype(bf16)
        wv = np.ascontiguousarray(np.concatenate(v_rows, 0).T).astype(bf16)
        wp = np.ascontiguousarray(W_proj[:, 256 * hg : 256 * (hg + 1)].T).astype(bf16)
        in_maps.append({"xt": xt, "wqk": wqk, "wv": wv, "wp": wp})
    return in_maps


def kernel(x, W_kqv, W_proj, b_proj):
    from concourse.bass_utils import run_bass_kernel_spmd

    nc = get_nc()
    in_maps = _shard_inputs(x, W_kqv, W_proj)
    res = run_bass_kernel_spmd(nc, in_maps, core_ids=list(range(8)))
    B = x.shape[0]
    out = np.empty((B, T, C), np.float32)
    for b in range(B):
        acc = res.results[4 * b]["y"].astype(np.float32).copy()
        for hg in range(1, 4):
            acc += res.results[4 * b + hg]["y"]
        out[b] = acc + b_proj[None, :]
    return out


# revision 6
# speedup vs baseline: 1.1026x; 1.0917x over previous
"""Causal self-attention kernel for 8 trn2 NeuronCores.

Sharding: core c handles batch b = c // 4 and local head group hg = c % 4
(4 of the 16 heads). Tensor-parallel over heads for kqv / attention and
row-parallel for the output projection; the 4 per-batch partial projections
are summed on the host (the "all-reduce" of classic TP), where the bias is
also added.

Device kernel (per core, bf16 matmuls, fp32 accumulation), interleaved over
tq windows g of 512 so PE / ScalarE / VectorE / DMA overlap end to end:
  kq(g):   kqT window = (Wqk x^T)[:, g]      [512 feat, 512 t]  k,q head-major
  v(g):    v chunks 4g..4g+3 = (x Wv^T)      per-head [128 t, 64] + 64-wide
           ones block (used to compute softmax denominators on the PE)
  attn(g): heads processed in pairs sharing one 1024-wide psum strip
           (h_even -> cols 0:512, h_odd -> cols 512:1024), tk chunks
           j <= 4g+3, causally trimmed at col cs:
             S^T = k^T.T q^T; the pair's two K=64 matmuls use PE row groups
             0-63 / 64-127 and run concurrently
             P = exp(S^T/8): ONE ScalarE activation covers both heads
             (no max subtraction -- scores are O(1) by construction)
             diagonal causal mask: affine_select on GpSimd (128x128 squares)
             O^T psum[0:64] += v_j.T @ P ; psum[64:128] += ones.T @ P (=denom)
           normalize: l -> SBUF, reciprocal_approx_fast, multiply (VectorE)
  proj(g-1): y[:, window] = O_cat^T.T @ Wp^T -> fp32, DMA out (deferred one
           window so the PE never waits on the normalize chain)
"""

import numpy as np
import ml_dtypes

T = 2048
C = 1024
NH_LOCAL = 4
D = 64
TQW = 512  # tq window width
NGRP = T // TQW  # 4 tq windows
NCHUNK = T // 128  # 16 tk chunks

_nc_cache = {}


def _build_bass():
    import concourse.mybir as mybir
    import concourse.tile as tile
    from concourse import bacc

    f32 = mybir.dt.float32
    bf16 = mybir.dt.bfloat16

    nc = bacc.Bacc(None, target_bir_lowering=False)
    xt_d = nc.dram_tensor("xt", [C, T], bf16, kind="ExternalInput")
    wqk_d = nc.dram_tensor("wqk", [C, 512], bf16, kind="ExternalInput")
    wv_d = nc.dram_tensor("wv", [C, 256], bf16, kind="ExternalInput")
    wp_d = nc.dram_tensor("wp", [256, C], bf16, kind="ExternalInput")
    y_d = nc.dram_tensor("y", [T, C], f32, kind="ExternalOutput")

    with tile.TileContext(nc) as tc:
        with (
            tc.tile_pool(name="persist", bufs=1) as pp,
            tc.tile_pool(name="mmp", bufs=2, space="PSUM") as mp,
            tc.tile_pool(name="spsum", bufs=2, space="PSUM") as sp,
            tc.tile_pool(name="opsum", bufs=1, space="PSUM") as op,
            tc.tile_pool(name="ptp", bufs=3) as ptp,
            tc.tile_pool(name="rp", bufs=4) as rp,
            tc.tile_pool(name="ysb", bufs=2) as ysb,
        ):
            xt_s = [pp.tile([128, T], bf16, tag=f"xt{c}", name=f"xt{c}") for c in range(8)]
            wqk_s = [pp.tile([128, 512], bf16, tag=f"wqk{c}", name=f"wqk{c}") for c in range(8)]
            wv_s = [pp.tile([128, 256], bf16, tag=f"wv{c}", name=f"wv{c}") for c in range(8)]
            wp_s = [pp.tile([128, C], bf16, tag=f"wp{c}", name=f"wp{c}") for c in range(2)]
            kq_s = [pp.tile([128, T], bf16, tag=f"kq{f}", name=f"kq{f}") for f in range(4)]
            v_s = [pp.tile([128, T], bf16, tag=f"v{h}", name=f"v{h}") for h in range(NH_LOCAL)]
            oc_s = [pp.tile([128, T], bf16, tag=f"oc{p}", name=f"oc{p}") for p in range(2)]

            # x(g=0) + qk weights first so kq(0) starts ASAP; wp last (needed
            # only at proj(0), deep into the kernel)
            for c in range(8):
                nc.sync.dma_start(wqk_s[c][:], wqk_d[128 * c : 128 * (c + 1), :])
            for c in range(8):
                nc.sync.dma_start(
                    xt_s[c][:, 0:TQW], xt_d[128 * c : 128 * (c + 1), 0:TQW]
                )
            for c in range(8):
                nc.sync.dma_start(wv_s[c][:], wv_d[128 * c : 128 * (c + 1), :])
            for g in range(1, NGRP):
                for c in range(8):
                    nc.sync.dma_start(
                        xt_s[c][:, TQW * g : TQW * (g + 1)],
                        xt_d[128 * c : 128 * (c + 1), TQW * g : TQW * (g + 1)],
                    )
            for p in range(2):
                nc.sync.dma_start(wp_s[p][:], wp_d[128 * p : 128 * (p + 1), :])

            # ones blocks for the denominator trick (v regions overwritten later)
            for h in range(NH_LOCAL):
                nc.vector.memset(v_s[h][:], 1.0)

            def proj(g):
                for i in range(4 * g, 4 * g + 4):
                    ys = ysb.tile([128, C], f32, tag="ys", name="ys")
                    for u in range(2):
                        acc = mp.tile([128, 512], f32, tag="mm", name="mmy")
                        for ci in range(2):
                            nc.tensor.matmul(
                                acc[:],
                                oc_s[ci][:, 128 * i : 128 * (i + 1)],
                                wp_s[ci][:, 512 * u : 512 * (u + 1)],
                                start=(ci == 0),
                                stop=(ci == 1),
                            )
                        nc.vector.tensor_copy(ys[:, 512 * u : 512 * (u + 1)], acc[:])
                    nc.sync.dma_start(y_d[128 * i : 128 * (i + 1), :], ys[:])

            for g in range(NGRP):
                w0 = TQW * g
                # ---- kq for window g ----
                for f in range(4):
                    acc = mp.tile([128, 512], f32, tag="mm", name="mmkq")
                    for c in range(8):
                        nc.tensor.matmul(
                            acc[:],
                            wqk_s[c][:, 128 * f : 128 * (f + 1)],
                            xt_s[c][:, w0 : w0 + TQW],
                            start=(c == 0),
                            stop=(c == 7),
                        )
                    nc.vector.tensor_copy(kq_s[f][:, w0 : w0 + TQW], acc[:])

                # ---- v chunks 4g..4g+3 ----
                for j in range(4 * g, 4 * g + 4):
                    acc = mp.tile([128, 512], f32, tag="mm", name="mmv")
                    for c in range(8):
                        nc.tensor.matmul(
                            acc[:, :256],
                            xt_s[c][:, 128 * j : 128 * (j + 1)],
                            wv_s[c][:],
                            start=(c == 0),
                            stop=(c == 7),
                        )
                    for h in range(NH_LOCAL):
                        nc.vector.tensor_copy(
                            v_s[h][:, 128 * j : 128 * j + 64],
                            acc[:, 64 * h : 64 * (h + 1)],
                        )

                # ---- attention for window g, head pairs share an S strip ----
                jmax = 4 * g + 3
                for hp in range(2):
                    h0, h1 = 2 * hp, 2 * hp + 1
                    o_t = {
                        h0: op.tile([128, TQW], f32, tag="oh0", name="oh0"),
                        h1: op.tile([128, TQW], f32, tag="oh1", name="oh1"),
                    }
                    for j in range(jmax + 1):
                        cs = max(0, 128 * j - w0)
                        s_t = sp.tile([128, 2 * TQW], f32, tag="s", name="s")
                        for idx, h in enumerate((h0, h1)):
                            kT = kq_s[h // 2][64 * (h % 2) : 64 * (h % 2) + 64, :]
                            qT = kq_s[2 + h // 2][64 * (h % 2) : 64 * (h % 2) + 64, :]
                            nc.tensor.matmul(
                                s_t[:, 512 * idx + cs : 512 * idx + 512],
                                kT[:, 128 * j : 128 * (j + 1)],
                                qT[:, w0 + cs : w0 + TQW],
                                start=True,
                                stop=True,
                            )
                        pt = ptp.tile([128, 2 * TQW], bf16, tag="pt", name="pt")
                        nc.scalar.activation(
                            pt[:, cs : 2 * TQW],
                            s_t[:, cs : 2 * TQW],
                            mybir.ActivationFunctionType.Exp,
                            scale=float(D) ** -0.5,
                        )
                        if 128 * j >= w0:
                            for idx in range(2):
                                nc.gpsimd.affine_select(
                                    out=pt[:, 512 * idx + cs : 512 * idx + cs + 128],
                                    in_=pt[:, 512 * idx + cs : 512 * idx + cs + 128],
                                    compare_op=mybir.AluOpType.is_ge,
                                    fill=0.0,
                                    base=0,
                                    pattern=[[1, 128]],
                                    channel_multiplier=-1,
                                )
                        for idx, h in enumerate((h0, h1)):
                            nc.tensor.matmul(
                                o_t[h][:, cs:TQW],
                                v_s[h][:, 128 * j : 128 * (j + 1)],
                                pt[:, 512 * idx + cs : 512 * idx + 512],
                                start=(j == 0),
                                stop=(j == jmax),
                            )
                    for h in (h0, h1):
                        lsb = rp.tile([64, 512], f32, tag="lsb", name="lsb")
                        rinv = rp.tile([64, 512], f32, tag="rinv", name="rinv")
                        nc.vector.tensor_copy(lsb[:], o_t[h][64:128, :])
                        nc.vector.reciprocal_approx_fast(rinv[:], lsb[:])
                        nc.vector.tensor_tensor(
                            oc_s[h // 2][
                                64 * (h % 2) : 64 * (h % 2) + 64, w0 : w0 + TQW
                            ],
                            o_t[h][0:64, :],
                            rinv[:],
                            mybir.AluOpType.mult,
                        )

                # ---- projection for the previous window ----
                if g > 0:
                    proj(g - 1)
            proj(NGRP - 1)

    nc.compile()
    return nc


def get_nc():
    if "nc" not in _nc_cache:
        _nc_cache["nc"] = _build_bass()
    return _nc_cache["nc"]


def _shard_inputs(x, W_kqv, W_proj):
    """Build the 8 per-core input maps (host-side shard + transpose + cast)."""
    bf16 = ml_dtypes.bfloat16
    in_maps = []
    for core in range(8):
        b, hg = core // 4, core % 4
        heads = range(4 * hg, 4 * hg + 4)
        xt = np.ascontiguousarray(x[b].T).astype(bf16)  # [C, T]
        k_rows = [W_kqv[64 * h : 64 * (h + 1)] for h in heads]
        q_rows = [W_kqv[C + 64 * h : C + 64 * (h + 1)] for h in heads]
        v_rows = [W_kqv[2 * C + 64 * h : 2 * C + 64 * (h + 1)] for h in heads]
        wqk = np.ascontiguousarray(np.concatenate(k_rows + q_rows, 0).T).astype(bf16)
        wv = np.ascontiguousarray(np.concatenate(v_rows, 0).T).astype(bf16)
        wp = np.ascontiguousarray(W_proj[:, 256 * hg : 256 * (hg + 1)].T).astype(bf16)
        in_maps.append({"xt": xt, "wqk": wqk, "wv": wv, "wp": wp})
    return in_maps


def kernel(x, W_kqv, W_proj, b_proj):
    from concourse.bass_utils import run_bass_kernel_spmd

    nc = get_nc()
    in_maps = _shard_inputs(x, W_kqv, W_proj)
    res = run_bass_kernel_spmd(nc, in_maps, core_ids=list(range(8)))
    B = x.shape[0]
    out = np.empty((B, T, C), np.float32)
    for b in range(B):
        acc = res.results[4 * b]["y"].astype(np.float32).copy()
        for hg in range(1, 4):
            acc += res.results[4 * b + hg]["y"]
        out[b] = acc + b_proj[None, :]
    return out
